# revision 2
# baseline (speedup 1.0000x reference)
"""Trainium2 Bass kernel for batched nearest-neighbor min-distance.

Problem: for each row u of U_z [16384, 256], compute
    min_{l in L_z [8192, 256]} ||u - l||_2
Strategy (8 NeuronCores, data-parallel over rows of U_z, L_z replicated;
`pred` is unused by the reference and ignored):
  d2(u,l) = ||u||^2 + ||l||^2 - 2 u.l
Per core (2048 U rows = "columns" of the transposed layout):
  - Inputs quantized to fp8 e4m3 on the host; the 256-dim contraction runs
    as ONE DoubleRow matmul per [128 L x 512 U] PSUM slab (the PE array
    virtualizes to 128x256 with 2 fp8 weights/cell => ~2x bf16 throughput).
    SBUF holds L^T as [128, 2, 8192] and (-2 U)^T as [128, 2, 2048] where
    dim d = ko*128 + p for (partition p, k-subtile ko).
  - Loop over 64 L-tiles (128 L rows each): PSUM[128 Lrows, 2048 Ucols]
    via 4 DoubleRow matmuls (N=512 each). Consumers fold in ||l||^2 - C
    and keep a running elementwise fp16 min, split across engines by a
    repeating tile pattern (mix):
      'A' tiles: ACT converts psum+bias -> fp16 (the only ACT pass), DVE
                 merges at fp16 2x into rmin.
      'D' tiles: DVE scalar_tensor_tensor min(psum+bias, rmin) directly
                 (1x fp32 PSUM read, but bias+min+consume in ONE op).
    Mix "AAAD" balances ACT (~1.85us/A-tile) against DVE
    (~1.1us merge/A-tile + ~2.3-2.7us/D-tile).
  - Partition reduction via DVE 32x32 block transpose + blocked free-dim
    min + two DMA-realigned tree levels, then add ||u||^2 + C, clamp at 0,
    sqrt, DMA out [32, 64] fp32 (column c = 32b + i at [i, b]).
The C=256 shift centers values so fp16 intermediates stay accurate; exact
(fp64) norms are used for the biases. fp8-quantization noise on the dot
products gives ~9e-3 max rel err on the final distances (vs 2e-2 budget;
validated in numpy on the harness's exact inputs).
"""

import numpy as np

N, M, D = 16384, 8192, 256
CORES = 8
C_SHIFT = 256.0
MIX = "AAAD"  # repeating consumer pattern over L-tiles: ACT+merge / DVE-direct

_COMPILED = {}


def _build(ucols: int, m: int, pattern=None, debug: bool = False, rounds: int = 1,
           warmup_mms: int = 0, conv_bufs: int = 8, mix: str = MIX,
           first_ch: int = 256, dma_ch: int = 1024, dma_split: bool = False):
    """Build + compile the per-core Bass kernel.

    ucols:  number of U columns (rows of U_z) this core handles.
    m:      number of L rows (library size).
    rounds: repeat the whole computation this many times inside a hardware
            loop (benchmarking only -- slope between round counts isolates
            steady-state HW time from the host dispatch overhead).
    pattern: benchmarking consumer-variant string (see baseline): 'X' = no
            consumer, 'A!' = ACT conv only, 'D!' = DVE stt only.
    """
    from contextlib import ExitStack, nullcontext

    import concourse.bacc as bacc
    import concourse.tile as tile
    from concourse import mybir

    F32 = mybir.dt.float32
    F16 = mybir.dt.float16
    FP8 = mybir.dt.float8e4
    AF = mybir.ActivationFunctionType
    ALU = mybir.AluOpType
    DR = mybir.MatmulPerfMode.DoubleRow

    ltiles = m // 128
    assert ucols % 512 == 0 and m % 128 == 0

    nc = bacc.Bacc("TRN2", target_bir_lowering=False, debug=debug)

    blocks = ucols // 32
    ut_d = nc.dram_tensor("ut", [128, 2, ucols], FP8, kind="ExternalInput").ap()
    lt_d = nc.dram_tensor("lt", [128, 2, m], FP8, kind="ExternalInput").ap()
    l2c_d = nc.dram_tensor("l2c", [128, ltiles], F32, kind="ExternalInput").ap()
    u2c_d = nc.dram_tensor("u2c", [32, blocks], F32, kind="ExternalInput").ap()
    out_d = nc.dram_tensor("out", [32, blocks], F32, kind="ExternalOutput").ap()

    with tile.TileContext(nc) as tc, ExitStack() as ctx:
        const_pool = ctx.enter_context(tc.tile_pool(name="const", bufs=1))
        psum_pool = ctx.enter_context(
            tc.tile_pool(name="psum", bufs=2, space="PSUM"))
        conv_pool = ctx.enter_context(tc.tile_pool(name="conv", bufs=conv_bufs))

        ut_sb = const_pool.tile([128, 2, ucols], FP8, name="utsb")
        lt_sb = const_pool.tile([128, 2, m], FP8, name="ltsb")
        l2c = const_pool.tile([128, ltiles], F32, name="l2c")
        u2c = const_pool.tile([32, blocks], F32, name="u2c")
        rmin16d = const_pool.tile([128, ucols], F16, name="rmin16d")

        wsrc = const_pool.tile([128, 512], FP8, name="wsrc")

        loop_cm = tc.For_i(0, rounds, 1) if rounds > 1 else nullcontext()
        ctx.enter_context(loop_cm)

        if pattern is not None:
            nc.vector.memset(rmin16d[:], 60000.0)
        if warmup_mms:
            # Dummy matmuls during the DMA head keep the PE HAM clock warm
            # (idle >3.4us re-throttles the PE to 1.2 GHz).
            nc.vector.memset(wsrc.bitcast(F32)[:], 1.0)
            wpsum = psum_pool.tile([128, ucols], F32, name="psum", tag="psum")
            for _ in range(warmup_mms):
                nc.tensor.matmul(wpsum[:, :512], wsrc[:, :128], wsrc[:],
                                 start=True, stop=True)

        # Small + U loads first so the main loop can start on L-chunk 0.
        nc.sync.dma_start(l2c[:], l2c_d[:])
        nc.sync.dma_start(u2c[:], u2c_d[:])
        nc.sync.dma_start(ut_sb[:], ut_d[:])
        chunks = [(0, first_ch)] if first_ch else []
        c0 = first_ch
        while c0 < m:
            ch = min(dma_ch, m - c0)
            chunks.append((c0, ch))
            c0 += ch
        for ci, (c0, ch) in enumerate(chunks):
            # dma_split: alternate the big L loads between the two
            # HWDGE engines (SP and ACT) for DMA queue parallelism.
            eng = nc.scalar if (dma_split and ci % 2) else nc.sync
            eng.dma_start(lt_sb[:, :, c0:c0 + ch], lt_d[:, :, c0:c0 + ch])

        for lt in range(ltiles):
            bias = l2c[:, lt:lt + 1]
            pat = pattern[lt % len(pattern)] if pattern is not None else ""
            psum = psum_pool.tile([128, ucols], F32, name="psum", tag="psum")
            lhsT = lt_sb[:, :, lt * 128:(lt + 1) * 128]
            for s0 in range(0, ucols, 512):
                nc.tensor.matmul(
                    psum[:, s0:s0 + 512],
                    lhsT,
                    ut_sb[:, :, s0:s0 + 512],
                    start=True,
                    stop=True,
                    perf_mode=DR,
                )
            if pat == "X":
                continue  # benchmarking variant: no consumer
            if pat == "A!":  # benchmarking: ACT conv only
                conva = conv_pool.tile([128, ucols], F16, name="conva",
                                       tag="conv")
                nc.scalar.activation(conva[:], psum[:], AF.Identity,
                                     bias=bias, scale=1.0)
                continue
            if pat == "D!":  # benchmarking: DVE fused min only
                nc.vector.scalar_tensor_tensor(
                    rmin16d[:], psum[:], bias,
                    rmin16d[:], op0=ALU.add, op1=ALU.min)
                continue
            # The first L-tile converts straight into rmin16d (no init
            # memset, no merge needed).
            if lt == 0:
                nc.scalar.activation(rmin16d[:], psum[:],
                                     AF.Identity, bias=bias, scale=1.0)
                continue
            if mix[lt % len(mix)] == "D":
                nc.vector.scalar_tensor_tensor(
                    rmin16d[:], psum[:], bias,
                    rmin16d[:], op0=ALU.add, op1=ALU.min)
                continue
            conv = conv_pool.tile([128, ucols], F16, name="conv", tag="conv")
            nc.scalar.activation(conv[:], psum[:], AF.Identity,
                                 bias=bias, scale=1.0)
            nc.vector.tensor_tensor(rmin16d[:], rmin16d[:], conv[:],
                                    op=ALU.min)

        fin = rmin16d
        # Partition reduction: transpose every 32x32 block of fin, min over
        # the free dim within each block -> red[32g + i, b] = min over
        # partitions {32g..32g+31} of column 32b + i. Then two tree levels
        # across the four partition groups (base partitions must be
        # 32-aligned and equal for DVE TT, so realign with tiny DMAs).
        tr = const_pool.tile([128, ucols], F16, name="tr")
        nc.vector.transpose(tr[:], fin[:])
        red = const_pool.tile([128, blocks], F16, name="red")
        nc.vector.tensor_reduce(
            red[:], tr.rearrange("p (b j) -> p b j", j=32),
            axis=mybir.AxisListType.X, op=ALU.min,
        )
        half = const_pool.tile([64, blocks], F16, name="half")
        nc.sync.dma_start(half[:], red[64:128, :])
        nc.vector.tensor_tensor(red[:64, :], red[:64, :], half[:, :], op=ALU.min)
        quart = const_pool.tile([32, blocks], F16, name="quart")
        nc.sync.dma_start(quart[:], red[32:64, :])
        nc.vector.tensor_tensor(red[:32, :], red[:32, :], quart[:, :], op=ALU.min)
        pmin = red[:32, :]
        d2 = const_pool.tile([32, blocks], F32, name="d2")
        nc.vector.tensor_tensor(d2[:], pmin[:], u2c[:], op=ALU.add)
        nc.vector.tensor_scalar_max(d2[:], d2[:], 0.0)
        outt = const_pool.tile([32, blocks], F32, name="outt")
        nc.scalar.activation(outt[:], d2[:], AF.Sqrt)
        nc.sync.dma_start(out_d[:], outt[:])

    nc.compile()
    return nc


def _get_compiled(ucols: int, m: int, **kwargs):
    key = (ucols, m, tuple(sorted(kwargs.items())))
    if key not in _COMPILED:
        _COMPILED[key] = _build(ucols, m, **kwargs)
    return _COMPILED[key]


def _prep_inputs(U: np.ndarray, L: np.ndarray):
    """Host-side sharding / layout prep (transpose, -2 scale, fp8 quantize,
    exact fp64 norms)."""
    import ml_dtypes

    FP8 = ml_dtypes.float8_e4m3fn
    n, d = U.shape
    m = L.shape[0]
    ucols = n // CORES
    # [p, ko, x] with contraction dim mapping d = ko*128 + p.
    UTm2 = np.ascontiguousarray(
        (-2.0 * U).T.reshape(2, 128, n).transpose(1, 0, 2).astype(FP8))
    LT = np.ascontiguousarray(
        L.T.reshape(2, 128, m).transpose(1, 0, 2).astype(FP8))
    l2 = (L.astype(np.float64) ** 2).sum(1).astype(np.float32)
    u2 = (U.astype(np.float64) ** 2).sum(1).astype(np.float32)
    l2cT = np.ascontiguousarray((l2 - C_SHIFT).reshape(m // 128, 128).T)
    u2c = u2 + C_SHIFT
    in_maps = []
    for i in range(CORES):
        sl = slice(i * ucols, (i + 1) * ucols)
        # Device output layout is [32, ucols//32] with column c = 32*b + i at
        # [i, b]; u2c must match that layout.
        u2c_dev = np.ascontiguousarray(u2c[sl].reshape(ucols // 32, 32).T)
        in_maps.append({
            "ut": np.ascontiguousarray(UTm2[:, :, sl]),
            "lt": LT,
            "l2c": l2cT,
            "u2c": u2c_dev,
        })
    return in_maps


def kernel(**inputs) -> np.ndarray:
    from concourse import bass_utils

    U = np.asarray(inputs["U_z"], dtype=np.float32)
    L = np.asarray(inputs["L_z"], dtype=np.float32)
    n = U.shape[0]
    m = L.shape[0]
    ucols = n // CORES
    nc = _get_compiled(ucols, m)
    in_maps = _prep_inputs(U, L)
    res = bass_utils.run_bass_kernel_spmd(nc, in_maps, list(range(CORES)))
    # Per-core output [32, ucols//32] holds column c = 32*b + i at [i, b].
    return np.concatenate(
        [np.ascontiguousarray(r["out"].T).reshape(-1) for r in res.results]
    ).astype(np.float32)


if __name__ == "__main__":
    # Smoke test with random data against a numpy reference.
    rng = np.random.default_rng(0)
    U = rng.standard_normal((N, D), dtype=np.float32)
    L = rng.standard_normal((M, D), dtype=np.float32)
    out = kernel(pred=None, U_z=U, L_z=L)
    d2 = (U * U).sum(1)[:, None] + (L * L).sum(1)[None, :] - 2.0 * U @ L.T
    exp = np.sqrt(np.maximum(d2, 0.0).min(1))
    rel = np.abs(out - exp) / np.maximum(np.abs(exp), 1e-9)
    print("max rel err:", rel.max())


# revision 3
# speedup vs baseline: 16.6431x; 16.6431x over previous
"""Trainium2 Bass kernel for batched nearest-neighbor min-distance.

Problem: for each row u of U_z [16384, 256], compute
    min_{l in L_z [8192, 256]} ||u - l||_2
Strategy (8 NeuronCores, data-parallel over rows of U_z, L_z replicated;
`pred` is unused by the reference and ignored):
  d2(u,l) = ||u||^2 + ||l||^2 - 2 u.l
Per core (2048 U rows = "columns" of the transposed layout):
  - Inputs quantized to fp8 e4m3 on the host; the 256-dim contraction runs
    as ONE DoubleRow matmul per [128 L x 512 U] PSUM slab (the PE array
    virtualizes to 128x256 with 2 fp8 weights/cell => ~2x bf16 throughput).
    SBUF holds L^T as [128, 2, 8192] and (-2 U)^T as [128, 2, 2048] where
    dim d = ko*128 + p for (partition p, k-subtile ko).
  - Loop over 64 L-tiles (128 L rows each): PSUM[128 Lrows, 2048 Ucols]
    via 4 DoubleRow matmuls (N=512 each). Consumers fold in ||l||^2 - C
    and keep a running elementwise fp16 min, split across engines by a
    repeating tile pattern (mix):
      'A' tiles: ACT converts psum+bias -> fp16 (the only ACT pass), DVE
                 merges at fp16 2x into rmin.
      'D' tiles: DVE scalar_tensor_tensor min(psum+bias, rmin) directly
                 (1x fp32 PSUM read, but bias+min+consume in ONE op).
    Mix "AAAD" balances ACT (~1.85us/A-tile) against DVE
    (~1.1us merge/A-tile + ~2.3-2.7us/D-tile).
  - Partition reduction via DVE 32x32 block transpose + blocked free-dim
    min + two DMA-realigned tree levels, then add ||u||^2 + C, clamp at 0,
    sqrt, DMA out [32, 64] fp32 (column c = 32b + i at [i, b]).
The C=256 shift centers values so fp16 intermediates stay accurate; exact
(fp64) norms are used for the biases. fp8-quantization noise on the dot
products gives ~9e-3 max rel err on the final distances (vs 2e-2 budget;
validated in numpy on the harness's exact inputs).
"""

import numpy as np

N, M, D = 16384, 8192, 256
CORES = 8
C_SHIFT = 256.0
MIX = "AAAD"  # repeating consumer pattern over L-tiles: ACT+merge / DVE-direct

_COMPILED = {}


def _build(ucols: int, m: int, pattern=None, debug: bool = False, rounds: int = 1,
           warmup_mms: int = 0, conv_bufs: int = 8, mix: str = MIX,
           first_ch: int = 256, dma_ch: int = 1024, dma_split: bool = False):
    """Build + compile the per-core Bass kernel.

    ucols:  number of U columns (rows of U_z) this core handles.
    m:      number of L rows (library size).
    rounds: repeat the whole computation this many times inside a hardware
            loop (benchmarking only -- slope between round counts isolates
            steady-state HW time from the host dispatch overhead).
    pattern: benchmarking consumer-variant string (see baseline): 'X' = no
            consumer, 'A!' = ACT conv only, 'D!' = DVE stt only.
    """
    from contextlib import ExitStack, nullcontext

    import concourse.bacc as bacc
    import concourse.tile as tile
    from concourse import mybir

    F32 = mybir.dt.float32
    F16 = mybir.dt.float16
    FP8 = mybir.dt.float8e4
    AF = mybir.ActivationFunctionType
    ALU = mybir.AluOpType
    DR = mybir.MatmulPerfMode.DoubleRow

    ltiles = m // 128
    assert ucols % 512 == 0 and m % 128 == 0

    nc = bacc.Bacc("TRN2", target_bir_lowering=False, debug=debug)

    blocks = ucols // 32
    ut_d = nc.dram_tensor("ut", [128, 2, ucols], FP8, kind="ExternalInput").ap()
    lt_d = nc.dram_tensor("lt", [128, 2, m], FP8, kind="ExternalInput").ap()
    l2c_d = nc.dram_tensor("l2c", [128, ltiles], F32, kind="ExternalInput").ap()
    u2c_d = nc.dram_tensor("u2c", [32, blocks], F32, kind="ExternalInput").ap()
    out_d = nc.dram_tensor("out", [32, blocks], F32, kind="ExternalOutput").ap()

    with tile.TileContext(nc) as tc, ExitStack() as ctx:
        const_pool = ctx.enter_context(tc.tile_pool(name="const", bufs=1))
        psum_pool = ctx.enter_context(
            tc.tile_pool(name="psum", bufs=2, space="PSUM"))
        conv_pool = ctx.enter_context(tc.tile_pool(name="conv", bufs=conv_bufs))

        ut_sb = const_pool.tile([128, 2, ucols], FP8, name="utsb")
        lt_sb = const_pool.tile([128, 2, m], FP8, name="ltsb")
        l2c = const_pool.tile([128, ltiles], F32, name="l2c")
        u2c = const_pool.tile([32, blocks], F32, name="u2c")
        rmin16d = const_pool.tile([128, ucols], F16, name="rmin16d")

        wsrc = const_pool.tile([128, 512], FP8, name="wsrc")

        loop_cm = tc.For_i(0, rounds, 1) if rounds > 1 else nullcontext()
        ctx.enter_context(loop_cm)

        if pattern is not None:
            nc.vector.memset(rmin16d[:], 60000.0)
        if warmup_mms:
            # Dummy matmuls during the DMA head keep the PE HAM clock warm
            # (idle >3.4us re-throttles the PE to 1.2 GHz).
            nc.vector.memset(wsrc.bitcast(F32)[:], 1.0)
            wpsum = psum_pool.tile([128, ucols], F32, name="psum", tag="psum")
            for _ in range(warmup_mms):
                nc.tensor.matmul(wpsum[:, :512], wsrc[:, :128], wsrc[:],
                                 start=True, stop=True)

        # Small + U loads first so the main loop can start on L-chunk 0.
        nc.sync.dma_start(l2c[:], l2c_d[:])
        nc.sync.dma_start(u2c[:], u2c_d[:])
        nc.sync.dma_start(ut_sb[:], ut_d[:])
        chunks = [(0, first_ch)] if first_ch else []
        c0 = first_ch
        while c0 < m:
            ch = min(dma_ch, m - c0)
            chunks.append((c0, ch))
            c0 += ch
        for ci, (c0, ch) in enumerate(chunks):
            # dma_split: alternate the big L loads between the two
            # HWDGE engines (SP and ACT) for DMA queue parallelism.
            eng = nc.scalar if (dma_split and ci % 2) else nc.sync
            eng.dma_start(lt_sb[:, :, c0:c0 + ch], lt_d[:, :, c0:c0 + ch])

        for lt in range(ltiles):
            bias = l2c[:, lt:lt + 1]
            pat = pattern[lt % len(pattern)] if pattern is not None else ""
            psum = psum_pool.tile([128, ucols], F32, name="psum", tag="psum")
            lhsT = lt_sb[:, :, lt * 128:(lt + 1) * 128]
            for s0 in range(0, ucols, 512):
                nc.tensor.matmul(
                    psum[:, s0:s0 + 512],
                    lhsT,
                    ut_sb[:, :, s0:s0 + 512],
                    start=True,
                    stop=True,
                    perf_mode=DR,
                )
            if pat == "X":
                continue  # benchmarking variant: no consumer
            if pat == "A!":  # benchmarking: ACT conv only
                conva = conv_pool.tile([128, ucols], F16, name="conva",
                                       tag="conv")
                nc.scalar.activation(conva[:], psum[:], AF.Identity,
                                     bias=bias, scale=1.0)
                continue
            if pat == "D!":  # benchmarking: DVE fused min only
                nc.vector.scalar_tensor_tensor(
                    rmin16d[:], psum[:], bias,
                    rmin16d[:], op0=ALU.add, op1=ALU.min)
                continue
            # The first L-tile converts straight into rmin16d (no init
            # memset, no merge needed).
            if lt == 0:
                nc.scalar.activation(rmin16d[:], psum[:],
                                     AF.Identity, bias=bias, scale=1.0)
                continue
            if mix[lt % len(mix)] == "D":
                nc.vector.scalar_tensor_tensor(
                    rmin16d[:], psum[:], bias,
                    rmin16d[:], op0=ALU.add, op1=ALU.min)
                continue
            conv = conv_pool.tile([128, ucols], F16, name="conv", tag="conv")
            nc.scalar.activation(conv[:], psum[:], AF.Identity,
                                 bias=bias, scale=1.0)
            if mix[lt % len(mix)] == "G":
                nc.gpsimd.tensor_tensor(rmin16d[:], rmin16d[:], conv[:],
                                        op=ALU.min)
            else:
                nc.vector.tensor_tensor(rmin16d[:], rmin16d[:], conv[:],
                                        op=ALU.min)

        fin = rmin16d
        # Partition reduction: transpose every 32x32 block of fin, min over
        # the free dim within each block -> red[32g + i, b] = min over
        # partitions {32g..32g+31} of column 32b + i. Then two tree levels
        # across the four partition groups (base partitions must be
        # 32-aligned and equal for DVE TT, so realign with tiny DMAs).
        tr = const_pool.tile([128, ucols], F16, name="tr")
        nc.vector.transpose(tr[:], fin[:])
        red = const_pool.tile([128, blocks], F16, name="red")
        nc.vector.tensor_reduce(
            red[:], tr.rearrange("p (b j) -> p b j", j=32),
            axis=mybir.AxisListType.X, op=ALU.min,
        )
        half = const_pool.tile([64, blocks], F16, name="half")
        nc.sync.dma_start(half[:], red[64:128, :])
        nc.vector.tensor_tensor(red[:64, :], red[:64, :], half[:, :], op=ALU.min)
        quart = const_pool.tile([32, blocks], F16, name="quart")
        nc.sync.dma_start(quart[:], red[32:64, :])
        nc.vector.tensor_tensor(red[:32, :], red[:32, :], quart[:, :], op=ALU.min)
        pmin = red[:32, :]
        d2 = const_pool.tile([32, blocks], F32, name="d2")
        nc.vector.tensor_tensor(d2[:], pmin[:], u2c[:], op=ALU.add)
        nc.vector.tensor_scalar_max(d2[:], d2[:], 0.0)
        outt = const_pool.tile([32, blocks], F32, name="outt")
        nc.scalar.activation(outt[:], d2[:], AF.Sqrt)
        nc.sync.dma_start(out_d[:], outt[:])

    nc.compile()
    return nc


def _get_compiled(ucols: int, m: int, **kwargs):
    key = (ucols, m, tuple(sorted(kwargs.items())))
    if key not in _COMPILED:
        _COMPILED[key] = _build(ucols, m, **kwargs)
    return _COMPILED[key]


def _prep_inputs(U: np.ndarray, L: np.ndarray):
    """Host-side sharding / layout prep (transpose, -2 scale, fp8 quantize,
    exact fp64 norms)."""
    import ml_dtypes

    FP8 = ml_dtypes.float8_e4m3fn
    n, d = U.shape
    m = L.shape[0]
    ucols = n // CORES
    # [p, ko, x] with contraction dim mapping d = ko*128 + p.
    UTm2 = np.ascontiguousarray(
        (-2.0 * U).T.reshape(2, 128, n).transpose(1, 0, 2).astype(FP8))
    LT = np.ascontiguousarray(
        L.T.reshape(2, 128, m).transpose(1, 0, 2).astype(FP8))
    l2 = (L.astype(np.float64) ** 2).sum(1).astype(np.float32)
    u2 = (U.astype(np.float64) ** 2).sum(1).astype(np.float32)
    l2cT = np.ascontiguousarray((l2 - C_SHIFT).reshape(m // 128, 128).T)
    u2c = u2 + C_SHIFT
    in_maps = []
    for i in range(CORES):
        sl = slice(i * ucols, (i + 1) * ucols)
        # Device output layout is [32, ucols//32] with column c = 32*b + i at
        # [i, b]; u2c must match that layout.
        u2c_dev = np.ascontiguousarray(u2c[sl].reshape(ucols // 32, 32).T)
        in_maps.append({
            "ut": np.ascontiguousarray(UTm2[:, :, sl]),
            "lt": LT,
            "l2c": l2cT,
            "u2c": u2c_dev,
        })
    return in_maps


def kernel(**inputs) -> np.ndarray:
    from concourse import bass_utils

    U = np.asarray(inputs["U_z"], dtype=np.float32)
    L = np.asarray(inputs["L_z"], dtype=np.float32)
    n = U.shape[0]
    m = L.shape[0]
    ucols = n // CORES
    nc = _get_compiled(ucols, m)
    in_maps = _prep_inputs(U, L)
    res = bass_utils.run_bass_kernel_spmd(nc, in_maps, list(range(CORES)))
    # Per-core output [32, ucols//32] holds column c = 32*b + i at [i, b].
    return np.concatenate(
        [np.ascontiguousarray(r["out"].T).reshape(-1) for r in res.results]
    ).astype(np.float32)


if __name__ == "__main__":
    # Smoke test with random data against a numpy reference.
    rng = np.random.default_rng(0)
    U = rng.standard_normal((N, D), dtype=np.float32)
    L = rng.standard_normal((M, D), dtype=np.float32)
    out = kernel(pred=None, U_z=U, L_z=L)
    d2 = (U * U).sum(1)[:, None] + (L * L).sum(1)[None, :] - 2.0 * U @ L.T
    exp = np.sqrt(np.maximum(d2, 0.0).min(1))
    rel = np.abs(out - exp) / np.maximum(np.abs(exp), 1e-9)
    print("max rel err:", rel.max())
